# revision 4
# baseline (speedup 1.0000x reference)
"""Trainium2 Bass kernel for AttentionBlock (B=8, C=256, L=2048), data-parallel
over batch across 8 NeuronCores.

Math (one batch per core, x: [C, L]):
    t^T   = w8^T x8            w8 = fp8(kappa M x),  M = Wq^T Wk,  kappa = 128*SCALE/ln2
    pT    = exp(t*ln2/128 + ux)  [m, l], keys m on partitions, fp8 direct from ACT
    denom = per-query sum of pT: cols 0:1024 via bf16 DVE accumulator + ones
            matmul, cols 1024:2048 via fp8 DoubleRow ones-matmuls accumulated
            in PSUM during the scores phase
    ctx   = vT^T pT in fp8 DoubleRow over chunk PAIRS (contracts 256 keys per
            instruction); vT = x^T Wv^T in fp8; ux rides as a 257th output
            column of the v projection
    out   = ctx * (1/denom) + (bf16(x) + bv)

All C- or key-contractions run fp8e4 DoubleRow ([128, 2, free] operands, 256
deep). On this silicon DR matches bf16 column rate (1 col/cycle), so its win
is halving instruction count for the 2048-deep key contractions (ctx, ds) and
enabling 256-deep channel contractions in one pass (scores, projections).

Schedule (v8):
  - exp runs as 2 x 1024-wide ACT instructions per chunk (vs 4 x 512): saves
    ~2 x 334 fixed cycles/chunk; ACT paces phase 2 at ~2.26us/chunk
  - pT is written fp8 directly by ACT; the DVE denominator accumulator reads
    fp8 at 1x, so it only covers query cols 0:1024; cols 1024:2048 accumulate
    on the PE (ones8 DR matmuls into 2 PSUM banks, one mm per key-pair)
  - PSUM: scores 2x[P,1024] (4 banks) + vp (1) + ds2/ds3 (2) + ctx(qt2,cc0)
    (1) = 8; the (2,0) ctx tile accumulating during phase 2 trims phase 3
  - phase 3: ds0/ds1 from the bf16 accumulator, 4 reciprocals, then 7
    remaining ctx tiles in fp8 DR (8 mms each) with evictions interleaved;
    last tile evicts in 4 sub-slices to shorten the DMA tail
  - w projection: 8 DR matmuls into 2 [P,2048] PSUM tiles; evict split
    ACT/DVE so the first 512 key-cols reach SBUF early and chunk 0 starts
    ~2us sooner than the old schedule; PE warms up on dummy matmuls into the
    same PSUM (overwritten via start=True)
  - residual prep pinned behind the denominator (baseline trick) so the
    scheduler cannot hoist it into the scores-phase DVE stream
"""

import math
import numpy as np
import ml_dtypes

import concourse.bass as bass
import concourse.tile as tile
from concourse import bacc, mybir
from concourse.bass_utils import run_bass_kernel_spmd

B, C, L = 8, 256, 2048
P = 128                 # partitions
NMC = L // P            # 16 m-chunks (key blocks)
NPAIR = NMC // 2        # 8 key pairs (256 keys each)
NB = 512                # matmul moving free dim
SCALE = float(C) ** -0.5
LN2 = math.log(2.0)
KAPPA = 128.0 * SCALE / LN2     # scores t = kappa * s_raw (baked into mt8 on host)
WARMUP_MMS = 4

F32 = mybir.dt.float32
BF16 = mybir.dt.bfloat16
F8 = mybir.dt.float8e4
DR = mybir.MatmulPerfMode.DoubleRow

_COMPILED = None


def build_nc():
    nc = bacc.Bacc("TRN2", target_bir_lowering=False, debug=False, num_devices=8)

    x8_d = nc.dram_tensor("x8", [C, L], F8, kind="ExternalInput").ap()
    xbf_d = nc.dram_tensor("xbf", [C, L], BF16, kind="ExternalInput").ap()
    mt8_d = nc.dram_tensor("mt8", [C, C], F8, kind="ExternalInput").ap()
    wvu8_d = nc.dram_tensor("wvu8", [C, 272], F8, kind="ExternalInput").ap()
    bv_d = nc.dram_tensor("bv", [C, 1], F32, kind="ExternalInput").ap()
    out_d = nc.dram_tensor("out", [C, L], BF16, kind="ExternalOutput").ap()

    with tile.TileContext(nc) as tc:
        with (
            tc.tile_pool(name="const", bufs=1) as const,
            tc.tile_pool(name="data", bufs=1) as data,
            tc.tile_pool(name="evict", bufs=6) as evict,
        ):
            # ---- constants / warmup fodder ----
            ones_bf = const.tile([P, NB], BF16)
            nc.vector.memset(ones_bf[:], 1.0)
            ones8 = const.tile([P, 2, P], F8)
            nc.gpsimd.memset(ones8[:], 1.0)
            tiny = const.tile([P, 2, 16], F32)

            x8 = data.tile([P, 2, L], F8, tag="x8", name="x8")
            xbf = [data.tile([P, L], BF16, tag=f"xbf{c}", name=f"xbf{c}")
                   for c in range(2)]
            mt8 = const.tile([P, 2, C], F8, tag="mt8")
            wvu8 = const.tile([P, 2, 272], F8, tag="wvu8")
            bv_sb = const.tile([P, 2, 1], F32, tag="bv")

            def x8_dma(c0, c1, eng):
                cols = slice(c0, c1)
                eng.dma_start(out=x8[:, :, cols],
                              in_=x8_d[:, cols].rearrange("(j p) l -> p j l",
                                                          p=P))

            # mt8 first (the w projection is the startup critical path),
            # then x8 slices in consumption order across all three queues
            nc.sync.dma_start(out=mt8[:],
                              in_=mt8_d.rearrange("(j p) o -> p j o", p=P))
            x8_dma(0, 512, nc.gpsimd)
            x8_dma(1024, 1536, nc.scalar)
            x8_dma(512, 1024, nc.sync)
            x8_dma(1536, 2048, nc.gpsimd)
            nc.scalar.dma_start(out=wvu8[:],
                                in_=wvu8_d.rearrange("(j p) o -> p j o", p=P))
            nc.scalar.dma_start(out=bv_sb[:],
                                in_=bv_d.rearrange("(j p) o -> p j o", p=P))

            w8 = data.tile([P, 2, L], F8, tag="w8", name="w8")
            vT8 = data.tile([P, NPAIR, 2, C], F8, tag="vT8")
            pT8 = data.tile([P, NPAIR, 2, L], F8, tag="pT8")
            b_act = data.tile([P, NMC, 1], F32, tag="b_act")
            bv_late = data.tile([P, 2, 1], F32, tag="bv_late")
            dacc = data.tile([P, 1024], BF16, tag="dacc")
            recip = data.tile([P, L], F32, tag="recip")
            xr = [data.tile([P, L], BF16, tag=f"xr{c}", name=f"xr{c}")
                  for c in range(2)]

            # ---- phase 1: PE warmup + w projection ----
            with tc.tile_pool(name="psA", bufs=1, space=bass.MemorySpace.PSUM) as psA:
                # warm the activation tables (one-time ~2.7us DMAs) and the PE
                # HAM clock-gate while x streams in; warmup matmuls write into
                # the w-projection tiles and are overwritten via start=True
                wp = [psA.tile([P, L], F32, tag=f"wp{oc}", name=f"wp{oc}")
                      for oc in range(2)]
                nc.vector.memset(tiny[:, 0, :], 1.0)
                nc.scalar.activation(out=tiny[:, 1, :], in_=tiny[:, 0, :],
                                     func=mybir.ActivationFunctionType.Exp,
                                     scale=1.0)
                nc.vector.reciprocal_approx_fast(out=tiny[:, 1, :],
                                                 in_=tiny[:, 0, :])
                for i in range(WARMUP_MMS):
                    nc.tensor.matmul(wp[i % 2][:, 0:NB], ones_bf[:, 0:P],
                                     ones_bf[:], start=True, stop=True)
                nc.tensor.matmul(wp[0][0:32, 0:16], ones8[:, :, 0:32],
                                 ones8[:, :, 0:16], start=True, stop=True,
                                 perf_mode=DR)

                # w = kappa M x (kappa baked into mt8 on host); one DoubleRow
                # matmul contracts the full 256 channels. Column-block order
                # so the first 512 key-cols are ready first.
                for b in range(4):
                    cols = slice(b * NB, (b + 1) * NB)
                    for oc in range(2):
                        nc.tensor.matmul(
                            wp[oc][:, cols],
                            mt8[:, :, oc * P:(oc + 1) * P],
                            x8[:, :, cols],
                            start=True, stop=True, perf_mode=DR)
                # evict split: ACT covers 0:512 (fast path for chunk 0) and
                # oc0 1024:2048; DVE covers 512:1024 and oc1 1024:2048
                nc.scalar.copy(out=w8[:, 0, 0:512], in_=wp[0][:, 0:512])
                nc.scalar.copy(out=w8[:, 1, 0:512], in_=wp[1][:, 0:512])
                nc.vector.tensor_copy(out=w8[:, 0, 512:1024],
                                      in_=wp[0][:, 512:1024])
                nc.vector.tensor_copy(out=w8[:, 1, 512:1024],
                                      in_=wp[1][:, 512:1024])
                nc.scalar.copy(out=w8[:, 0, 1024:2048], in_=wp[0][:, 1024:2048])
                nc.vector.tensor_copy(out=w8[:, 1, 1024:2048],
                                      in_=wp[1][:, 1024:2048])

            # xbf for the residual - only needed by the epilogue; these queues
            # are idle during the scores phase
            nc.sync.dma_start(out=xbf[0][:], in_=xbf_d[0:P, :])
            nc.gpsimd.dma_start(out=xbf[1][:], in_=xbf_d[P:C, :])

            # ---- phase 2: v-proj + scores + exp + denom + ctx(2,0) ----
            with tc.tile_pool(name="psCL", bufs=1,
                              space=bass.MemorySpace.PSUM) as psCL:
                ctx20 = psCL.tile([P, NB], F32, tag="c20", name="c20", bufs=1)
                ds23 = [psCL.tile([P, NB], F32, tag=f"ds{q}", name=f"ds{q}",
                                  bufs=1) for q in (2, 3)]

                with tc.tile_pool(name="psB", bufs=1,
                                  space=bass.MemorySpace.PSUM) as psB:
                    for mc in range(NMC):
                        pair, par = mc // 2, mc % 2
                        mrows = slice(mc * P, (mc + 1) * P)
                        # v/ux projection for this key chunk
                        vp = psB.tile([P, 272], F32, tag="vp", name="vp", bufs=1)
                        nc.tensor.matmul(
                            vp[:], x8[:, :, mrows], wvu8[:],
                            start=True, stop=True, perf_mode=DR)
                        # scores, two 1024-wide tiles (2 x 512 mms each)
                        s = [psB.tile([P, 1024], F32, tag="s", name="s",
                                      bufs=2) for _ in range(2)]
                        for h in range(2):
                            for ln in range(2):
                                q0 = h * 1024 + ln * NB
                                nc.tensor.matmul(
                                    s[h][:, ln * NB:(ln + 1) * NB],
                                    w8[:, :, mrows],
                                    x8[:, :, q0:q0 + NB],
                                    start=True, stop=True, perf_mode=DR)
                        # per-key exp bias (ux); vT must also free the vp bank
                        nc.vector.tensor_copy(out=b_act[:, mc, :],
                                              in_=vp[:, C:C + 1])
                        nc.vector.tensor_copy(out=vT8[:, pair, par, :],
                                              in_=vp[:, 0:C])
                        # exp -> fp8 pT, 1024 cols per instruction
                        for h in range(2):
                            nc.scalar.activation(
                                out=pT8[:, pair, par, h * 1024:(h + 1) * 1024],
                                in_=s[h][:],
                                func=mybir.ActivationFunctionType.Exp,
                                scale=LN2 / 128.0, bias=b_act[:, mc, :])
                        # running denominator for query cols 0:1024 (bf16 on
                        # DVE; fp8 source reads at 1x)
                        src = pT8[:, pair, par, 0:1024]
                        if mc == 0:
                            nc.vector.tensor_copy(out=dacc[:], in_=src)
                        else:
                            nc.vector.tensor_add(dacc[:], dacc[:], src)
                        if par == 1:
                            # pair complete: PE-side denominator for query
                            # cols 1024:2048 and the (2,0) ctx tile
                            st = (pair == 0)
                            sp = (pair == NPAIR - 1)
                            for q in range(2):
                                qcols = slice(1024 + q * NB, 1024 + (q + 1) * NB)
                                nc.tensor.matmul(
                                    ds23[q][:], ones8[:],
                                    pT8[:, pair, :, qcols],
                                    start=st, stop=sp, perf_mode=DR)
                            nc.tensor.matmul(
                                ctx20[:], vT8[:, pair, :, 0:P],
                                pT8[:, pair, :, 1024:1536],
                                start=st, stop=sp, perf_mode=DR)

                # ---- phase 3: denom finish + ctx-right + epilogue ----
                with tc.tile_pool(name="psDR", bufs=1,
                                  space=bass.MemorySpace.PSUM) as psDR:
                    def ctx_mms(ct, qt, cc):
                        for pr in range(NPAIR):
                            nc.tensor.matmul(
                                ct[:],
                                vT8[:, pr, :, cc * P:(cc + 1) * P],
                                pT8[:, pr, :, qt * NB:(qt + 1) * NB],
                                start=(pr == 0), stop=(pr == NPAIR - 1),
                                perf_mode=DR)

                    def ct_evict(ct, qt, cc, nsub, qpick):
                        rows = slice(cc * P, (cc + 1) * P)
                        sub = NB // nsub
                        for si in range(nsub):
                            c0 = qt * NB + si * sub
                            cols = slice(c0, c0 + sub)
                            pcols = slice(si * sub, (si + 1) * sub)
                            t = evict.tile([P, sub], F32, tag="t", name="t")
                            nc.vector.tensor_mul(t[:], ct[:, pcols],
                                                 recip[:, cols])
                            o = evict.tile([P, sub], BF16, tag="o", name="o")
                            nc.gpsimd.tensor_add(o[:], t[:], xr[cc][:, cols])
                            deng = (nc.sync, nc.scalar,
                                    nc.gpsimd)[(qpick + si) % 3]
                            deng.dma_start(out=out_d[rows, cols], in_=o[:])

                    # finish the denominator: qt0/qt1 from the bf16
                    # accumulator, qt2/qt3 straight from the PSUM tiles
                    ds01 = []
                    for q in range(2):
                        cols = slice(q * NB, (q + 1) * NB)
                        ds = psDR.tile([P, NB], F32, tag="ds", name="ds",
                                       bufs=2)
                        ds01.append(ds)
                        nc.tensor.matmul(ds[:], ones_bf[:, 0:P],
                                         dacc[:, cols],
                                         start=True, stop=True)
                    nc.vector.reciprocal_approx_fast(out=recip[:, 1024:1536],
                                                     in_=ds23[0][:])
                    nc.vector.reciprocal_approx_fast(out=recip[:, 1536:2048],
                                                     in_=ds23[1][:])
                    nc.vector.reciprocal_approx_fast(out=recip[:, 0:512],
                                                     in_=ds01[0][:])
                    nc.vector.reciprocal_approx_fast(out=recip[:, 512:1024],
                                                     in_=ds01[1][:])
                    # residual prep, pinned behind the denominator so the
                    # scheduler cannot hoist it into the scores-phase DVE queue
                    nc.vector.tensor_scalar(out=bv_late[:], in0=bv_sb[:],
                                            scalar1=ds01[0][:, 0:1],
                                            scalar2=ds01[0][:, 0:1],
                                            op0=mybir.AluOpType.add,
                                            op1=mybir.AluOpType.subtract)
                    for cc in range(2):
                        nc.vector.tensor_scalar_add(out=xr[cc][:],
                                                    in0=xbf[cc][:],
                                                    scalar1=bv_late[:, cc, :])
                    ctxR = {}
                    order = ((2, 1), (3, 0), (3, 1), (0, 0), (0, 1),
                             (1, 0), (1, 1))
                    evict_after = {
                        (2, 1): [((2, 0), 1, 0)],
                        (3, 0): [((2, 1), 1, 1)],
                        (3, 1): [((3, 0), 1, 2)],
                        (0, 0): [((3, 1), 1, 0)],
                        (0, 1): [((0, 0), 1, 1)],
                        (1, 0): [((0, 1), 1, 2)],
                        (1, 1): [((1, 0), 2, 0)],
                    }
                    all_tiles = {(2, 0): ctx20}
                    for qt, cc in order:
                        ct = psDR.tile([P, NB], F32, tag="cr", name="cr",
                                       bufs=3)
                        ctxR[(qt, cc)] = ct
                        all_tiles[(qt, cc)] = ct
                        ctx_mms(ct, qt, cc)
                        for (eqt, ecc), nsub, qpick in evict_after[(qt, cc)]:
                            ct_evict(all_tiles[(eqt, ecc)], eqt, ecc, nsub,
                                     qpick)
                    ct_evict(ctxR[(1, 1)], 1, 1, 4, 2)

    nc.compile()
    return nc


def get_compiled():
    global _COMPILED
    if _COMPILED is None:
        _COMPILED = build_nc()
    return _COMPILED


def make_in_maps(inputs):
    f8 = ml_dtypes.float8_e4m3
    x = np.ascontiguousarray(np.asarray(inputs["x"], dtype=np.float32))
    Wq = np.asarray(inputs["Wq"], np.float32)
    Wk = np.asarray(inputs["Wk"], np.float32)
    Wv = np.asarray(inputs["Wv"], np.float32)
    bq = np.asarray(inputs["bq"], np.float32)
    M = Wq.T @ Wk                               # scores_raw = x^T M x
    u = SCALE * (Wk.T @ bq)                     # per-key score bias u.x
    wvu = np.zeros((C, 272), np.float32)
    wvu[:, 0:C] = Wv.T
    wvu[:, C] = u
    shared = {
        "mt8": np.ascontiguousarray(KAPPA * M.T).astype(f8),
        "wvu8": wvu.astype(f8),
        "bv": np.asarray(inputs["bv"], np.float32).reshape(C, 1),
    }
    return [{"x8": x[i].astype(f8), "xbf": x[i].astype(ml_dtypes.bfloat16),
             **shared} for i in range(B)]


def run(inputs, trace=False, **kwargs):
    nc = get_compiled()
    res = run_bass_kernel_spmd(nc, make_in_maps(inputs),
                               core_ids=list(range(B)), trace=trace, **kwargs)
    out = np.stack([res.results[i]["out"] for i in range(B)], axis=0)
    return out.astype(np.float32), res


def kernel(**inputs):
    out, _ = run(inputs)
    return out
